# revision 1
# baseline (speedup 1.0000x reference)
"""Causal multi-head self-attention with RoPE on 8 Trainium2 NeuronCores.

Sharding: tensor-parallel over heads — core c owns heads (2c, 2c+1) for BOTH
batch elements.  Everything on-chip is computed "transposed" (feature dim on
partitions, tokens on the free dim):

  phase A  qT/kT/vT = W @ x^T per core (K=1024 contraction, f32r matmuls);
           RoPE applied to qT/kT elementwise with on-device cos/sin tiles
           (built from token_positions via range-reduced ACT Sin);
           vT transposed back to (token, dim) tiles via PE transpose into a
           130-column-per-ktile layout with a ones column per head (the ones
           column makes the AV matmul emit the softmax denominator for free).
  phase B  per (batch, q-chunk 512, k-tile 128):
             logitsT (k-part, q-free) = kT_h^T @ qT_h — two heads row-tiled
             concurrently (array rows 0-63 / 64-127) into one psum tile;
             e = exp(logits/8) (no max subtraction; logits are O(1));
             ctxT (65, q) += [v | 1]^T @ e accumulated over k-tiles.
           causality: k-tiles above the diagonal are skipped, diagonal tiles
           get restricted q-ranges plus one 128x128 triangular mask multiply.
           Emitted per batch right after that batch's phase A so batch 0's
           attention overlaps batch 1's projections.
  phase C  normalize ctx rows by the summed row, stage per token-quarter,
           one 8-core AllToAll so core d ends with ALL 1024 ctx dims for
           (batch d//4, token quarter d%4), local out-projection with wo^T.
           Each core returns (512, 1024); the host concatenates.
"""
import os
import sys

import numpy as np

for p in ("/opt/trn_rl_repo", "/root/.axon_site/_ro/trn_rl_repo"):
    if os.path.isdir(p) and p not in sys.path:
        sys.path.insert(0, p)

D_MODEL = 1024
NUM_HEADS = 16
D_K = 64
THETA = 10000.0
BATCH = 2
SEQ = 2048
NCORES = 8
H_PER_CORE = 2
DIMS = H_PER_CORE * D_K   # 128 ctx dims owned per core
S2 = BATCH * SEQ          # 4096 token columns (both batches)
QC = 512                  # q-chunk
KT = 128                  # k-tile
MAGIC = 3 * 2.0**22       # fp32 round-to-nearest-integer trick
SCALE = 0.125             # 1/sqrt(d_k)

_CACHE = {}


def _build_program():
    import concourse.mybir as mybir
    import concourse.tile as tile
    from concourse import bacc
    from concourse.masks import make_identity, make_upper_triangular
    from concourse.tile import add_dep_helper

    F32 = mybir.dt.float32
    F32R = mybir.dt.float32r
    F16 = mybir.dt.float16
    I32 = mybir.dt.int32
    AFT = mybir.ActivationFunctionType

    nc = bacc.Bacc("TRN2", target_bir_lowering=False, debug=False,
                   num_devices=NCORES)

    xT_d = nc.declare_dram_parameter("xT", [D_MODEL, S2], F16, isOutput=False)
    wqT_d = nc.declare_dram_parameter("wqT", [D_MODEL, DIMS], F16, isOutput=False)
    wkT_d = nc.declare_dram_parameter("wkT", [D_MODEL, DIMS], F16, isOutput=False)
    wvT_d = nc.declare_dram_parameter("wvT", [D_MODEL, DIMS], F16, isOutput=False)
    woT_d = nc.declare_dram_parameter("woT", [D_MODEL, D_MODEL], F16, isOutput=False)
    pos_d = nc.declare_dram_parameter("pos", [1, S2], I32, isOutput=False)
    invf_d = nc.declare_dram_parameter("invf", [1, DIMS], F32, isOutput=False)
    out_d = nc.declare_dram_parameter("out", [QC, D_MODEL], F32, isOutput=True)

    with tile.TileContext(nc) as tc:
        with tc.tile_pool(name="consts", bufs=1) as consts, \
             tc.tile_pool(name="qkr", bufs=2) as qkr, \
             tc.tile_pool(name="vbufp", bufs=2) as vbufp, \
             tc.tile_pool(name="ps", bufs=1, space="PSUM") as ps, \
             tc.tile_pool(name="epool", bufs=3) as epool, \
             tc.tile_pool(name="stp", bufs=2) as stp, \
             tc.tile_pool(name="rrp", bufs=2) as rrp, \
             tc.tile_pool(name="dram", bufs=1, space="DRAM") as dram:

            # ---------- small constants ----------
            tri_f = consts.tile([KT, KT], F32)
            make_upper_triangular(nc, tri_f[:], val=1.0, diag=True)
            tri_r = consts.tile([KT, KT], F32R)
            nc.vector.tensor_copy(tri_r, tri_f)
            ident = consts.tile([128, 128], F32)
            make_identity(nc, ident[:])
            ones16 = consts.tile([128, 16], F32)
            nc.vector.memset(ones16, 1.0)
            invf_t = consts.tile([1, DIMS], F32)
            nc.sync.dma_start(out=invf_t, in_=invf_d[:])

            a2a_in = dram.tile([NCORES, DIMS, QC], F16, name="a2ain")
            a2a_out = dram.tile([NCORES, DIMS, QC], F16, name="a2aout")

            qR, kR, vbuf = {}, {}, {}
            first_exp = {}

            with tc.tile_pool(name="phA", bufs=1) as phA, \
                 tc.tile_pool(name="xtp", bufs=8) as xtp, \
                 tc.tile_pool(name="wp", bufs=1) as wp:

                pos_f = phA.tile([1, S2], F32, tag="posf")
                pos_i = phA.tile([1, S2], I32, tag="posi")
                nc.sync.dma_start(out=pos_i, in_=pos_d[:])
                nc.vector.tensor_copy(pos_f, pos_i)

                w_sb = {}
                for nm, d in (("q", wqT_d), ("k", wkT_d), ("v", wvT_d)):
                    wt = wp.tile([128, 8, DIMS], F16, tag=f"w{nm}",
                                 name=f"w{nm}")
                    nc.sync.dma_start(
                        out=wt,
                        in_=d[:].rearrange("(e p) c -> p e c", p=128))
                    w_sb[nm] = [wt[:, k8, :] for k8 in range(8)]

                # ---- cos/sin tiles for BOTH batches (emitted after
                # batch 0's projections so the angle matmuls don't head-of-
                # line block the PE queue; single Sin table-set residency) ----
                CSb = {}
                sin_insts = []

                def emit_cs():
                    for b in range(BATCH):
                        CSb[b] = {"S": phA.tile([DIMS, SEQ], F32, tag=f"csS{b}",
                                                name=f"csS{b}"),
                                  "C": phA.tile([DIMS, SEQ], F32, tag=f"csC{b}",
                                                name=f"csC{b}")}
                        for quarter in range(4):
                            sl = slice(quarter * 512, (quarter + 1) * 512)
                            gsl = slice(b * SEQ + quarter * 512,
                                        b * SEQ + (quarter + 1) * 512)
                            pang = ps.tile([DIMS, 512], F32, tag="m1", bufs=2,
                                           name="pang")
                            nc.tensor.matmul(pang, invf_t, pos_f[:, gsl],
                                             start=True, stop=True)
                            tq = phA.tile([DIMS, 512], F32, tag="csb_t", name="tq")
                            nc.scalar.copy(tq, pang)
                            for kind, bias in (("S", 0.0), ("C", 0.25)):
                                tb = phA.tile([DIMS, 512], F32, tag="csb_b",
                                              name="tb", bufs=2)
                                if bias:
                                    nc.vector.tensor_scalar_add(tb, tq, bias)
                                else:
                                    nc.vector.tensor_copy(tb, tq)
                                tm = phA.tile([DIMS, 512], F32, tag="csb_m",
                                              name="tm", bufs=2)
                                nc.vector.tensor_scalar_add(tm, tb, MAGIC)
                                nc.vector.tensor_scalar_sub(tm, tm, MAGIC)
                                nc.vector.tensor_sub(tb, tb, tm)
                                sin_insts.append(nc.scalar.activation(
                                    CSb[b][kind][:, sl], tb, AFT.Sin,
                                    scale=2 * np.pi))

                for b in range(BATCH):
                    # ---- q/k/v projections, 512-token chunks ----
                    raw = {nm: phA.tile([DIMS, SEQ], F32, tag=f"raw{nm}",
                                        name=f"raw{nm}")
                           for nm in ("q", "k", "v")}
                    for th2 in range(SEQ // 1024):
                        xt = []
                        for k8 in range(8):
                            t = xtp.tile([128, 1024], F16, tag="xt", name="xt")
                            nc.sync.dma_start(
                                out=t,
                                in_=xT_d[k8 * 128:(k8 + 1) * 128,
                                         b * SEQ + th2 * 1024:
                                         b * SEQ + (th2 + 1) * 1024])
                            xt.append(t)
                        for half in range(2):
                            th = th2 * 2 + half
                            hsl2 = slice(half * 512, (half + 1) * 512)
                            for nm in ("q", "k", "v"):
                                pp = ps.tile([DIMS, 512], F32, tag="proj",
                                             bufs=2, name="pp")
                                for k8 in range(8):
                                    nc.tensor.matmul(pp, w_sb[nm][k8],
                                                     xt[k8][:, hsl2],
                                                     start=(k8 == 0),
                                                     stop=(k8 == 7))
                                nc.scalar.copy(
                                    raw[nm][:, th * 512:(th + 1) * 512], pp)

                    if b == 0:
                        emit_cs()
                    CS = CSb[b]

                    # ---- RoPE on q, k (token halves) ----
                    for nm in ("q", "k"):
                        rot = qkr.tile([DIMS, SEQ], F32R, tag=f"{nm}R",
                                       name=f"{nm}R{b}")
                        for hh in range(2):
                            hsl = slice(hh * 1024, (hh + 1) * 1024)
                            src = raw[nm][:, hsl].rearrange(
                                "(p two) n -> p two n", two=2)
                            swp = phA.tile([DIMS, 1024], F32, tag="swap",
                                           name="swp")
                            dst = swp[:].rearrange("(p two) n -> p two n", two=2)
                            nc.sync.dma_start(out=dst[:, 0, :], in_=src[:, 1, :])
                            nc.sync.dma_start(out=dst[:, 1, :], in_=src[:, 0, :])
                            t1 = phA.tile([DIMS, 1024], F32, tag="ropet1",
                                          name="t1")
                            nc.vector.tensor_mul(t1, raw[nm][:, hsl],
                                                 CS["C"][:, hsl])
                            nc.vector.tensor_mul(swp, swp, CS["S"][:, hsl])
                            nc.gpsimd.tensor_add(rot[:, hsl], t1, swp)
                        (qR if nm == "q" else kR)[b] = rot

                    # ---- v -> (token, dim) tiles with ones columns ----
                    vb = vbufp.tile([128, 130 * (SEQ // KT)], F32R, tag="vbuf",
                                    name=f"vbuf{b}")
                    vbuf[b] = vb
                    v_view = vb[:].rearrange("p (t c) -> p t c", c=130)
                    for col in (64, 129):
                        nc.vector.tensor_copy(
                            v_view[:, :, col:col + 1].rearrange(
                                "p t c -> p (t c)"),
                            ones16)
                    for t16 in range(SEQ // KT):
                        ptp = ps.tile([128, 128], F32, tag="proj", bufs=2,
                                      name="ptp")
                        nc.tensor.transpose(
                            ptp, raw["v"][:, t16 * 128:(t16 + 1) * 128], ident[:])
                        nc.vector.tensor_copy(
                            vb[:, 130 * t16:130 * t16 + 64], ptp[:, 0:64])
                        nc.vector.tensor_copy(
                            vb[:, 130 * t16 + 65:130 * t16 + 129], ptp[:, 64:128])

                    # ---- attention for this batch (overlaps next batch's
                    # phase A via the scheduler) ----
                    for qc in range(SEQ // QC):
                        pctx = [ps.tile([65, QC], F32, tag=f"ctx{h}", bufs=1,
                                        name=f"pctx{h}")
                                for h in range(H_PER_CORE)]
                        nkt = 4 * qc + 4

                        def emit_m2(kt, q0, et, nkt=nkt, b=b, pctx=pctx):
                            for h in range(H_PER_CORE):
                                vt = vbuf[b][:, 130 * kt + 65 * h:
                                             130 * kt + 65 * h + 65]
                                nc.tensor.matmul(
                                    pctx[h][:, q0:QC], vt,
                                    et[:, h * QC + q0:(h + 1) * QC],
                                    start=(kt == 0), stop=(kt == nkt - 1),
                                    skip_group_check=True)

                        pend = []
                        for kt in range(nkt):
                            j = kt - 4 * qc
                            q0 = 0 if j < 0 else KT * j
                            pl = ps.tile([128, 2 * QC], F32, tag="m1", bufs=2,
                                         name="pl")
                            for h in range(H_PER_CORE):
                                nc.tensor.matmul(
                                    pl[:, h * QC + q0:(h + 1) * QC],
                                    kR[b][64 * h:64 * (h + 1),
                                          kt * KT:(kt + 1) * KT],
                                    qR[b][64 * h:64 * (h + 1),
                                          qc * QC + q0:(qc + 1) * QC],
                                    start=True, stop=True)
                            et = epool.tile([128, 2 * QC], F32R, tag="e",
                                            name="et")
                            if q0 == 0:
                                ei = nc.scalar.activation(et, pl, AFT.Exp,
                                                          scale=SCALE)
                                if (b, qc) not in first_exp:
                                    first_exp[(b, qc)] = ei
                                    add_dep_helper(ei.ins, sin_insts[-1].ins,
                                                   sync=True,
                                                   reason="sin set before exp")
                            else:
                                ev = et[:].rearrange("p (h n) -> p h n", h=2)[
                                    :, :, q0:QC]
                                pv = pl[:].rearrange("p (h n) -> p h n", h=2)[
                                    :, :, q0:QC]
                                nc.scalar.activation(ev, pv, AFT.Exp,
                                                     scale=SCALE)
                            if j >= 0:
                                for h in range(H_PER_CORE):
                                    msl = slice(h * QC + q0, h * QC + q0 + KT)
                                    nc.vector.tensor_mul(et[:, msl], et[:, msl],
                                                         tri_r)
                            pend.append((kt, q0, et))
                            if len(pend) > 2:
                                emit_m2(*pend.pop(0))
                        for p2 in pend:
                            emit_m2(*p2)

                        # normalize + stage for the A2A
                        stage = stp.tile([128, QC], F16, tag="stage",
                                         name="stage")
                        for h in range(H_PER_CORE):
                            r = rrp.tile([1, QC], F32, tag="r", name="r")
                            nc.vector.reciprocal(r, pctx[h][64:65, :])
                            R = rrp.tile([64, QC], F32, tag="R", name="R")
                            nc.gpsimd.partition_broadcast(R[:], r[:])
                            nc.vector.tensor_mul(
                                stage[64 * h:64 * (h + 1), :],
                                pctx[h][0:64, :], R)
                        nc.sync.dma_start(out=a2a_in[4 * b + qc], in_=stage)

            # ---------- phase C pools (reuse phase-A space) ----------
            with tc.tile_pool(name="wop", bufs=1) as wop, \
                 tc.tile_pool(name="a2ap", bufs=1) as a2ap, \
                 tc.tile_pool(name="outp", bufs=2) as outp:

                wo_sb = []
                for k8 in range(8):
                    t = wop.tile([128, D_MODEL], F16, tag=f"wo{k8}",
                                 name=f"wo{k8}")
                    nc.sync.dma_start(out=t, in_=woT_d[k8 * 128:(k8 + 1) * 128, :])
                    wo_sb.append(t)

                # ---------- one AllToAll + output projection ----------
                nc.gpsimd.collective_compute(
                    "AllToAll", mybir.AluOpType.bypass,
                    replica_groups=[list(range(NCORES))],
                    ins=[a2a_in.opt()], outs=[a2a_out.opt()],
                )
                cmerged = a2ap.tile([DIMS, NCORES, QC], F16, tag="a2a",
                                    name="a2a")
                nc.sync.dma_start(
                    out=cmerged,
                    in_=a2a_out[:].rearrange("e p c -> p e c"))
                ctx_sb = [cmerged[:, i, :] for i in range(NCORES)]
                for mt in range(QC // 128):
                    ot = outp.tile([128, D_MODEL], F32, tag="out", name="ot")
                    for nn in range(2):
                        po = ps.tile([128, 512], F32, tag="proj", bufs=2,
                                     name="po")
                        for i in range(NCORES):
                            nc.tensor.matmul(
                                po, ctx_sb[i][:, mt * 128:(mt + 1) * 128],
                                wo_sb[i][:, nn * 512:(nn + 1) * 512],
                                start=(i == 0), stop=(i == NCORES - 1))
                        # ACT is idle after attention; keep DVE free
                        nc.scalar.copy(ot[:, nn * 512:(nn + 1) * 512], po)
                    nc.sync.dma_start(out=out_d[mt * 128:(mt + 1) * 128, :],
                                      in_=ot)

    nc.compile()
    return nc


def _host_prep(inputs):
    x = np.asarray(inputs["in_features"], dtype=np.float32)
    tp = np.asarray(inputs["token_positions"], dtype=np.int32)
    wq = np.asarray(inputs["wq"], dtype=np.float32)
    wk = np.asarray(inputs["wk"], dtype=np.float32)
    wv = np.asarray(inputs["wv"], dtype=np.float32)
    wo = np.asarray(inputs["wo"], dtype=np.float32)

    xT = np.ascontiguousarray(
        np.concatenate([x[b].T for b in range(BATCH)], axis=1)).astype(np.float16)
    woT = np.ascontiguousarray(wo.T).astype(np.float16)
    pos = np.ascontiguousarray(tp.reshape(1, S2))

    # signed inv-freq in turns: within-head dim d: freq j = d//2,
    # sign -1 on even rows (the S tile row becomes -sin), +1 on odd rows.
    j = (np.arange(DIMS) % D_K) // 2
    sign = np.where(np.arange(DIMS) % 2 == 0, -1.0, 1.0)
    invf = (sign / (THETA ** (2.0 * j / D_K)) / (2 * np.pi)).astype(np.float32)
    invf = np.ascontiguousarray(invf.reshape(1, DIMS))

    in_maps = []
    for c in range(NCORES):
        rows = slice(DIMS * c, DIMS * (c + 1))
        in_maps.append({
            "xT": xT,
            "wqT": np.ascontiguousarray(wq[rows].T).astype(np.float16),
            "wkT": np.ascontiguousarray(wk[rows].T).astype(np.float16),
            "wvT": np.ascontiguousarray(wv[rows].T).astype(np.float16),
            "woT": woT,
            "pos": pos,
            "invf": invf,
        })
    return in_maps


def kernel(**inputs) -> np.ndarray:
    from concourse.bass_utils import run_bass_kernel_spmd

    if "nc" not in _CACHE:
        _CACHE["nc"] = _build_program()
    nc = _CACHE["nc"]

    in_maps = _host_prep(inputs)
    res = run_bass_kernel_spmd(nc, in_maps, list(range(NCORES))).results

    out = np.empty((BATCH, SEQ, D_MODEL), dtype=np.float32)
    for c in range(NCORES):
        b, quarter = c // 4, c % 4
        out[b, quarter * QC:(quarter + 1) * QC, :] = res[c]["out"]
    return out



# revision 16
# speedup vs baseline: 1.0266x; 1.0266x over previous
"""Causal multi-head self-attention with RoPE on 8 Trainium2 NeuronCores.

Sharding: tensor-parallel over heads — core c owns heads (2c, 2c+1) for BOTH
batch elements.  Feature dim lives on partitions, tokens on the free dim.

  phase A  (software-pipelined per 512-token chunk, per batch)
           qT/kT/vT = W @ x^T (f16 matmuls, K=1024 contraction);
           RoPE on qT/kT with HOST-precomputed cos/sin f16 tables
           (rot = x*C + swap(x)*S with the sign folded into S);
           vT transposed to (token, dim) 130-col k-tile layout with a ones
           column per head (denominator comes free out of the AV matmul).
  phase B  per (batch, q-chunk 512) — emitted with a 1-chunk skew so chunk
           t+1's projections hide chunk t's RoPE latency:
             logitsT (k-part, q-free) f16 = kT_h^T @ qT_h, 2 heads packed in
             one [128, 1024] psum; e = exp(logits/8) -> f16;
             AV runs transposed: ctx[q-part, 65] += e_blk^T @ [v | 1], one
             65-wide matmul per (head, q-tile 128, k-tile) — only q-tiles on
             or below the diagonal. Diagonal k-tiles get one 128x128
             triangular mask multiply per head.
           normalize by the ones-column (per-partition scalar on Pool),
           PE-transpose back to (dim, token), stage as f16.
  phase C  per batch: one 8-core AllToAll (512 KB) exchanging half-chunks:
           core d gets tokens [qc*512 + half*256, +256) of batch b where
           qc = d//2, half = d%2, with ALL 1024 ctx dims; local out-
           projection with wo^T.  Batch 0's AllToAll and projection overlap
           batch 1's compute; only batch 1's sits in the tail.
           Each core returns (2, 256, 1024); the host concatenates.
"""
import os
import sys

import numpy as np

for p in ("/opt/trn_rl_repo", "/root/.axon_site/_ro/trn_rl_repo"):
    if os.path.isdir(p) and p not in sys.path:
        sys.path.insert(0, p)

D_MODEL = 1024
NUM_HEADS = 16
D_K = 64
THETA = 10000.0
BATCH = 2
SEQ = 2048
NCORES = 8
H_PER_CORE = 2
DIMS = H_PER_CORE * D_K   # 128 ctx dims owned per core
QC = 512                  # q-chunk
KT = 128                  # k-tile
SCALE = 0.125             # 1/sqrt(d_k)

_CACHE = {}


def _build_program():
    import concourse.mybir as mybir
    import concourse.tile as tile
    from concourse import bacc
    from concourse.masks import make_identity, make_upper_triangular

    F32 = mybir.dt.float32
    F16 = mybir.dt.float16
    AFT = mybir.ActivationFunctionType

    nc = bacc.Bacc("TRN2", target_bir_lowering=False, debug=False,
                   num_devices=NCORES)

    xT_d = nc.declare_dram_parameter("xT", [D_MODEL, BATCH * SEQ], F16,
                                     isOutput=False)
    wqT_d = nc.declare_dram_parameter("wqT", [D_MODEL, DIMS], F16, isOutput=False)
    wkT_d = nc.declare_dram_parameter("wkT", [D_MODEL, DIMS], F16, isOutput=False)
    wvT_d = nc.declare_dram_parameter("wvT", [D_MODEL, DIMS], F16, isOutput=False)
    woT_d = nc.declare_dram_parameter("woT", [D_MODEL, D_MODEL], F16, isOutput=False)
    csC_d = nc.declare_dram_parameter("csC", [DIMS, BATCH * SEQ], F16,
                                      isOutput=False)
    csS_d = nc.declare_dram_parameter("csS", [DIMS, BATCH * SEQ], F16,
                                      isOutput=False)
    out_d = nc.declare_dram_parameter("out", [BATCH, 2 * KT, D_MODEL], F32,
                                      isOutput=True)
    DEBUG = bool(os.environ.get("K_DEBUG"))
    if DEBUG:
        dbg_qr = nc.declare_dram_parameter("dbg_qr", [DIMS, SEQ], F16,
                                           isOutput=True)
        dbg_kr = nc.declare_dram_parameter("dbg_kr", [DIMS, SEQ], F16,
                                           isOutput=True)
        dbg_vb = nc.declare_dram_parameter("dbg_vb", [128, 130 * (SEQ // KT)],
                                           F16, isOutput=True)
        dbg_st = nc.declare_dram_parameter("dbg_st", [NCORES, DIMS, 2 * KT],
                                           F16, isOutput=True)
        dbg_px = nc.declare_dram_parameter("dbg_px", [128, QC], F32,
                                           isOutput=True)
        dbg_et = nc.declare_dram_parameter("dbg_et", [2, 128, 2 * QC], F32,
                                           isOutput=True)

    NCH = SEQ // QC           # 4 chunks per batch
    NVT = QC // KT            # 4 k-tiles per chunk

    with tile.TileContext(nc) as tc:
        with tc.tile_pool(name="consts", bufs=1) as consts, \
             tc.tile_pool(name="qk", bufs=1) as qkp, \
             tc.tile_pool(name="vbufp", bufs=1) as vbufp, \
             tc.tile_pool(name="ps", bufs=1, space="PSUM") as ps, \
             tc.tile_pool(name="epool", bufs=17) as epool, \
             tc.tile_pool(name="rawp", bufs=2) as rawp, \
             tc.tile_pool(name="xtp", bufs=16) as xtp, \
             tc.tile_pool(name="ropep", bufs=2) as ropep, \
             tc.tile_pool(name="normp", bufs=4) as normp, \
             tc.tile_pool(name="stp", bufs=2) as stp, \
             tc.tile_pool(name="wp", bufs=1) as wp, \
             tc.tile_pool(name="outp", bufs=2) as outp, \
             tc.tile_pool(name="dram", bufs=1, space="DRAM") as dram:

            # ---------- constants ----------
            tri_f = consts.tile([KT, KT], F32)
            make_upper_triangular(nc, tri_f[:], val=1.0, diag=True)
            tri = consts.tile([KT, KT], F16)
            nc.vector.tensor_copy(tri, tri_f)
            ident = consts.tile([128, 128], F32)
            make_identity(nc, ident[:])
            ones16 = consts.tile([128, 16], F16)
            nc.vector.memset(ones16, 1.0)

            # cos/sin tables for both batches (host-precomputed)
            csC = consts.tile([DIMS, BATCH * SEQ], F16, name="csC")
            csS = consts.tile([DIMS, BATCH * SEQ], F16, name="csS")
            nc.sync.dma_start(out=csC, in_=csC_d[:])
            nc.sync.dma_start(out=csS, in_=csS_d[:])

            a2a_in = [dram.tile([NCORES, DIMS, 2 * KT], F16, name=f"a2ain{b}")
                      for b in range(BATCH)]
            a2a_out = [dram.tile([NCORES, DIMS, 2 * KT], F16, name=f"a2aout{b}")
                       for b in range(BATCH)]

            # per-batch persistent tiles
            qR = {b: qkp.tile([DIMS, SEQ], F16, tag=f"qR{b}", name=f"qR{b}")
                  for b in range(BATCH)}
            kR = {b: qkp.tile([DIMS, SEQ], F16, tag=f"kR{b}", name=f"kR{b}")
                  for b in range(BATCH)}
            vbuf = {b: vbufp.tile([128, 130 * (SEQ // KT)], F16, tag=f"vb{b}",
                                  name=f"vbuf{b}")
                    for b in range(BATCH)}

            # projection weights
            w_sb = {}
            for nm, d in (("q", wqT_d), ("k", wkT_d), ("v", wvT_d)):
                wt = wp.tile([128, 8, DIMS], F16, tag=f"w{nm}", name=f"w{nm}")
                nc.sync.dma_start(
                    out=wt, in_=d[:].rearrange("(e p) c -> p e c", p=128))
                w_sb[nm] = [wt[:, k8, :] for k8 in range(8)]

            def emit_wo_loads():
                wo_sb = []
                for k8 in range(8):
                    t = wp.tile([128, D_MODEL], F16, tag=f"wo{k8}",
                                name=f"wo{k8}")
                    nc.sync.dma_start(out=t,
                                      in_=woT_d[k8 * 128:(k8 + 1) * 128, :])
                    wo_sb.append(t)
                return wo_sb

            # ---------- phase A: one 512-token chunk ----------
            def emit_chunk(b, t):
                g0 = b * SEQ + t * QC          # global column offset
                c0 = t * QC                    # within-batch column offset
                xt = []
                for k8 in range(8):
                    xtile = xtp.tile([128, QC], F16, tag="xt", name="xt")
                    nc.sync.dma_start(
                        out=xtile,
                        in_=xT_d[k8 * 128:(k8 + 1) * 128, g0:g0 + QC])
                    xt.append(xtile)
                raw = {}
                for nm in ("q", "k", "v"):
                    pp = ps.tile([128, QC], F32, tag="m1", bufs=2, name="pp")
                    for k8 in range(8):
                        nc.tensor.matmul(pp, w_sb[nm][k8], xt[k8],
                                         start=(k8 == 0), stop=(k8 == 7))
                    # v stays f32: the PE transpose output dtype must match
                    # its input, and the psum pool is f32
                    r = rawp.tile([DIMS, QC], F16 if nm != "v" else F32,
                                  tag=f"raw{nm}", name=f"raw{nm}")
                    nc.scalar.copy(r, pp)
                    raw[nm] = r

                # v -> (token, dim) tiles with ones columns
                vb = vbuf[b]
                v_view = vb[:, 130 * NVT * t:130 * NVT * (t + 1)].rearrange(
                    "p (n c) -> p n c", c=130)
                for col in (64, 129):
                    nc.vector.tensor_copy(
                        v_view[:, :, col:col + 1].rearrange("p n c -> p (n c)"),
                        ones16[:, 0:NVT])
                for i in range(NVT):
                    ptp = ps.tile([128, QC], F32, tag="m1", bufs=2, name="ptp")
                    nc.tensor.transpose(ptp[:, 0:128],
                                        raw["v"][:, i * 128:(i + 1) * 128],
                                        ident[:])
                    # both head-halves in one strided copy (skips ones cols)
                    dst = vb[:, 130 * (NVT * t + i):130 * (NVT * t + i) + 130]
                    dst = dst.rearrange("p (h c) -> p h c", h=2)[:, :, 0:64]
                    src = ptp[:, 0:128].rearrange("p (h c) -> p h c", h=2)
                    nc.vector.tensor_copy(dst, src)

                # RoPE: rot = x*C + swap(x)*S   (sign baked into S rows)
                for nm in ("q", "k"):
                    src = raw[nm][:].rearrange("(p two) n -> p two n", two=2)
                    swp = ropep.tile([DIMS, QC], F16, tag="swap", name="swp")
                    dstv = swp[:].rearrange("(p two) n -> p two n", two=2)
                    nc.sync.dma_start(out=dstv[:, 0, :], in_=src[:, 1, :])
                    nc.sync.dma_start(out=dstv[:, 1, :], in_=src[:, 0, :])
                    t1 = ropep.tile([DIMS, QC], F16, tag="t1", name="t1")
                    nc.vector.tensor_mul(t1, raw[nm], csC[:, g0:g0 + QC])
                    nc.vector.tensor_mul(swp, swp, csS[:, g0:g0 + QC])
                    dst = (qR if nm == "q" else kR)[b][:, c0:c0 + QC]
                    nc.vector.tensor_add(dst, t1, swp)

            # ---------- phase B: one q-chunk of attention ----------
            def emit_attn(b, qc):
                pctx = [ps.tile([128, QC], F32, tag=f"ctx{h}", bufs=1,
                                name=f"pctx{h}")
                        for h in range(H_PER_CORE)]
                nkt = NVT * qc + NVT
                ets = []
                for kt in range(nkt):
                    j = kt - NVT * qc            # >=0: diagonal band tile
                    q0 = 0 if j < 0 else KT * j
                    pl = ps.tile([128, 2 * QC], F32, tag="logit", bufs=2,
                                 name="pl")
                    for h in range(H_PER_CORE):
                        nc.tensor.matmul(
                            pl[:, h * QC + q0:(h + 1) * QC],
                            kR[b][64 * h:64 * (h + 1), kt * KT:(kt + 1) * KT],
                            qR[b][64 * h:64 * (h + 1),
                                  qc * QC + q0:(qc + 1) * QC],
                            start=True, stop=True)
                    et = epool.tile([128, 2 * QC], F16, tag="e", name="et")
                    if q0 == 0:
                        nc.scalar.activation(et, pl, AFT.Exp, scale=SCALE)
                    else:
                        ev = et[:].rearrange("p (h n) -> p h n", h=2)[
                            :, :, q0:QC]
                        pv = pl[:].rearrange("p (h n) -> p h n", h=2)[
                            :, :, q0:QC]
                        nc.scalar.activation(ev, pv, AFT.Exp, scale=SCALE)
                    if j >= 0:
                        for h in range(H_PER_CORE):
                            msl = slice(h * QC + q0, h * QC + q0 + KT)
                            nc.vector.tensor_mul(et[:, msl], et[:, msl], tri)
                    if DEBUG and b == 0 and qc == 0 and kt <= 1:
                        dt_ = normp.tile([128, 2 * QC], F32, tag="dbge",
                                         name="dbge")
                        nc.vector.tensor_copy(dt_, et)
                        nc.sync.dma_start(out=dbg_et[kt], in_=dt_)
                    ets.append(et)
                # AV, one accumulation group per (h, q-tile): only one open
                # group per psum bank at a time (PE constraint)
                for qt in range(NVT):
                    for kt in range(NVT * qc + qt + 1):
                        for h in range(H_PER_CORE):
                            vt = vbuf[b][:, 130 * kt + 65 * h:
                                         130 * kt + 65 * h + 65]
                            nc.tensor.matmul(
                                pctx[h][:, qt * KT:qt * KT + 65],
                                ets[kt][:, h * QC + qt * KT:
                                        h * QC + (qt + 1) * KT],
                                vt,
                                start=(kt == 0),
                                stop=(kt == NVT * qc + qt),
                                skip_group_check=True)

                if DEBUG and b == 0 and qc == 0:
                    dpx = normp.tile([128, QC], F32, tag="dbgp", name="dbgp")
                    nc.vector.tensor_copy(dpx, pctx[0][:])
                    nc.sync.dma_start(out=dbg_px[:], in_=dpx)

                # normalize, transpose to (dim, token), stage for the A2A
                recips = []
                for h in range(H_PER_CORE):
                    rc = normp.tile([128, NVT], F32, tag="rc", name="rc")
                    den = pctx[h][:].rearrange("p (n c) -> p n c", c=KT)[
                        :, :, 64:65].rearrange("p n c -> p (n c)")
                    nc.vector.reciprocal(rc, den)
                    recips.append(rc)
                for qt in range(NVT):
                    tp = ps.tile([128, QC], F32, tag="m1", bufs=2, name="tp")
                    nr = normp.tile([128, 128], F32, tag="nr", name="nr")
                    for h in range(H_PER_CORE):
                        nc.vector.tensor_scalar_mul(
                            nr[:, 64 * h:64 * (h + 1)],
                            pctx[h][:, qt * KT:qt * KT + 64],
                            recips[h][:, qt:qt + 1])
                    nc.tensor.transpose(tp[:, 0:128], nr, ident[:])
                    stg = stp.tile([128, KT], F16, tag="stage", name="stage")
                    nc.scalar.copy(stg, tp[:, 0:128])
                    nc.sync.dma_start(
                        out=a2a_in[b][2 * qc + qt // 2, :,
                                      (qt % 2) * KT:(qt % 2) * KT + KT],
                        in_=stg)

            def emit_a2a(b):
                nc.gpsimd.collective_compute(
                    "AllToAll", mybir.AluOpType.bypass,
                    replica_groups=[list(range(NCORES))],
                    ins=[a2a_in[b].opt()], outs=[a2a_out[b].opt()],
                )

            def emit_woproj(b, wo_sb):
                cm = wp.tile([DIMS, NCORES, 2 * KT], F16, tag=f"cm{b}",
                             name=f"cm{b}")
                nc.sync.dma_start(
                    out=cm, in_=a2a_out[b][:].rearrange("e p c -> p e c"))
                for mt in range(2):
                    ot = outp.tile([128, D_MODEL], F32, tag="out", name="ot")
                    for nn in range(2):
                        po = ps.tile([128, QC], F32, tag="m1", bufs=2,
                                     name="po")
                        for i in range(NCORES):
                            nc.tensor.matmul(
                                po, cm[:, i, mt * KT:(mt + 1) * KT],
                                wo_sb[i][:, nn * QC:(nn + 1) * QC],
                                start=(i == 0), stop=(i == NCORES - 1))
                        nc.scalar.copy(ot[:, nn * QC:(nn + 1) * QC], po)
                    nc.sync.dma_start(out=out_d[b, mt * KT:(mt + 1) * KT, :],
                                      in_=ot)

            # ---------- emission schedule ----------
            emit_chunk(0, 0)
            wo_sb = emit_wo_loads()
            emit_chunk(0, 1)
            emit_attn(0, 0)
            emit_chunk(0, 2)
            emit_attn(0, 1)
            emit_chunk(0, 3)
            emit_attn(0, 2)
            emit_chunk(1, 0)
            emit_attn(0, 3)
            emit_a2a(0)
            emit_chunk(1, 1)
            emit_attn(1, 0)
            emit_chunk(1, 2)
            emit_attn(1, 1)
            emit_chunk(1, 3)
            emit_attn(1, 2)
            emit_woproj(0, wo_sb)
            emit_attn(1, 3)
            emit_a2a(1)
            emit_woproj(1, wo_sb)
            if DEBUG:
                nc.sync.dma_start(out=dbg_qr[:], in_=qR[0])
                nc.sync.dma_start(out=dbg_kr[:], in_=kR[0])
                nc.sync.dma_start(out=dbg_vb[:], in_=vbuf[0])
                cm0 = wp.tile([DIMS, NCORES, 2 * KT], F16, tag="cm0",
                              name="cmdbg")
                nc.sync.dma_start(
                    out=cm0, in_=a2a_out[0][:].rearrange("e p c -> p e c"))
                nc.sync.dma_start(
                    out=dbg_st[:].rearrange("e p c -> p e c"), in_=cm0)

    nc.compile()
    return nc


def _host_prep(inputs):
    x = np.asarray(inputs["in_features"], dtype=np.float32)
    tp = np.asarray(inputs["token_positions"], dtype=np.int32)
    wq = np.asarray(inputs["wq"], dtype=np.float32)
    wk = np.asarray(inputs["wk"], dtype=np.float32)
    wv = np.asarray(inputs["wv"], dtype=np.float32)
    wo = np.asarray(inputs["wo"], dtype=np.float32)

    xT = np.ascontiguousarray(
        np.concatenate([x[b].T for b in range(BATCH)], axis=1)).astype(np.float16)
    woT = np.ascontiguousarray(wo.T).astype(np.float16)

    # cos/sin tables, (dim row, batch*token col); sign baked into S so that
    # rot = x*C + swap(x)*S
    half = D_K // 2
    inv_freq = 1.0 / (THETA ** (2.0 * np.arange(half) / D_K))     # (32,)
    ang = tp.astype(np.float64)[:, :, None] * inv_freq[None, None, :]
    cos = np.cos(ang)                                             # (B, S, 32)
    sin = np.sin(ang)
    rows = np.arange(DIMS)
    j = (rows % D_K) // 2                                         # freq index
    sign = np.where(rows % 2 == 0, -1.0, 1.0)
    csC = np.empty((DIMS, BATCH * SEQ), dtype=np.float16)
    csS = np.empty((DIMS, BATCH * SEQ), dtype=np.float16)
    for b in range(BATCH):
        csC[:, b * SEQ:(b + 1) * SEQ] = cos[b][:, j].T
        csS[:, b * SEQ:(b + 1) * SEQ] = (sin[b][:, j] * sign[None, :]).T

    in_maps = []
    for c in range(NCORES):
        rsl = slice(DIMS * c, DIMS * (c + 1))
        in_maps.append({
            "xT": xT,
            "wqT": np.ascontiguousarray(wq[rsl].T).astype(np.float16),
            "wkT": np.ascontiguousarray(wk[rsl].T).astype(np.float16),
            "wvT": np.ascontiguousarray(wv[rsl].T).astype(np.float16),
            "woT": woT,
            "csC": csC,
            "csS": csS,
        })
    return in_maps


def kernel(**inputs) -> np.ndarray:
    from concourse.bass_utils import run_bass_kernel_spmd

    if "nc" not in _CACHE:
        _CACHE["nc"] = _build_program()
    nc = _CACHE["nc"]

    in_maps = _host_prep(inputs)
    res = run_bass_kernel_spmd(nc, in_maps, list(range(NCORES))).results

    out = np.empty((BATCH, SEQ, D_MODEL), dtype=np.float32)
    for c in range(NCORES):
        qc, hf = c // 2, c % 2
        t0 = qc * QC + hf * 2 * KT
        for b in range(BATCH):
            out[b, t0:t0 + 2 * KT, :] = res[c]["out"][b]
    return out


# revision 22
# speedup vs baseline: 1.0723x; 1.0445x over previous
"""Causal multi-head self-attention with RoPE on 8 Trainium2 NeuronCores.

Sharding: tensor-parallel over heads — core c owns heads (2c, 2c+1) for BOTH
batch elements.  Feature dim lives on partitions, tokens on the free dim.

  phase A  (software-pipelined per 512-token chunk, per batch)
           qT/kT/vT = W @ x^T (f16 matmuls, K=1024 contraction);
           RoPE on qT/kT with HOST-precomputed cos/sin f16 tables
           (rot = x*C + swap(x)*S with the sign folded into S);
           vT transposed to (token, dim) 130-col k-tile layout with a ones
           column per head (denominator comes free out of the AV matmul).
  phase B  per (batch, q-chunk 512) — emitted with a 1-chunk skew so chunk
           t+1's projections hide chunk t's RoPE latency:
             logitsT (k-part, q-free) f16 = kT_h^T @ qT_h, 2 heads packed in
             one [128, 1024] psum; e = exp(logits/8) -> f16;
             AV runs transposed: ctx[q-part, 65] += e_blk^T @ [v | 1], one
             65-wide matmul per (head, q-tile 128, k-tile) — only q-tiles on
             or below the diagonal. Diagonal k-tiles get one 128x128
             triangular mask multiply per head.
           normalize by the ones-column (per-partition scalar on Pool),
           PE-transpose back to (dim, token), stage as f16.
  phase C  per batch: one 8-core AllToAll (512 KB) exchanging half-chunks:
           core d gets tokens [qc*512 + half*256, +256) of batch b where
           qc = d//2, half = d%2, with ALL 1024 ctx dims; local out-
           projection with wo^T.  Batch 0's AllToAll and projection overlap
           batch 1's compute; only batch 1's sits in the tail.
           Each core returns (2, 256, 1024); the host concatenates.
"""
import os
import sys

import numpy as np

for p in ("/opt/trn_rl_repo", "/root/.axon_site/_ro/trn_rl_repo"):
    if os.path.isdir(p) and p not in sys.path:
        sys.path.insert(0, p)

D_MODEL = 1024
NUM_HEADS = 16
D_K = 64
THETA = 10000.0
BATCH = 2
SEQ = 2048
NCORES = 8
H_PER_CORE = 2
DIMS = H_PER_CORE * D_K   # 128 ctx dims owned per core
QC = 512                  # q-chunk
KT = 128                  # k-tile
SCALE = 0.125             # 1/sqrt(d_k)

_CACHE = {}


def _build_program():
    import concourse.mybir as mybir
    import concourse.tile as tile
    from concourse import bacc
    from concourse.masks import make_identity, make_upper_triangular

    F32 = mybir.dt.float32
    F16 = mybir.dt.float16
    AFT = mybir.ActivationFunctionType

    nc = bacc.Bacc("TRN2", target_bir_lowering=False, debug=False,
                   num_devices=NCORES)

    xT_d = nc.declare_dram_parameter("xT", [D_MODEL, BATCH * SEQ], F16,
                                     isOutput=False)
    wqT_d = nc.declare_dram_parameter("wqT", [D_MODEL, DIMS], F16, isOutput=False)
    wkT_d = nc.declare_dram_parameter("wkT", [D_MODEL, DIMS], F16, isOutput=False)
    wvT_d = nc.declare_dram_parameter("wvT", [D_MODEL, DIMS], F16, isOutput=False)
    woT_d = nc.declare_dram_parameter("woT", [D_MODEL, D_MODEL], F16, isOutput=False)
    csC_d = nc.declare_dram_parameter("csC", [DIMS, BATCH * SEQ], F16,
                                      isOutput=False)
    csS_d = nc.declare_dram_parameter("csS", [DIMS, BATCH * SEQ], F16,
                                      isOutput=False)
    out_d = nc.declare_dram_parameter("out", [BATCH, 2 * KT, D_MODEL], F32,
                                      isOutput=True)
    DEBUG = bool(os.environ.get("K_DEBUG"))
    if DEBUG:
        dbg_qr = nc.declare_dram_parameter("dbg_qr", [DIMS, SEQ], F16,
                                           isOutput=True)
        dbg_kr = nc.declare_dram_parameter("dbg_kr", [DIMS, SEQ], F16,
                                           isOutput=True)
        dbg_vb = nc.declare_dram_parameter("dbg_vb", [128, 130 * (SEQ // KT)],
                                           F16, isOutput=True)
        dbg_st = nc.declare_dram_parameter("dbg_st", [NCORES, DIMS, 2 * KT],
                                           F16, isOutput=True)
        dbg_px = nc.declare_dram_parameter("dbg_px", [128, QC], F32,
                                           isOutput=True)
        dbg_et = nc.declare_dram_parameter("dbg_et", [2, 128, 2 * QC], F32,
                                           isOutput=True)

    NCH = SEQ // QC           # 4 chunks per batch
    NVT = QC // KT            # 4 k-tiles per chunk

    with tile.TileContext(nc) as tc:
        with tc.tile_pool(name="consts", bufs=1) as consts, \
             tc.tile_pool(name="qk", bufs=1) as qkp, \
             tc.tile_pool(name="vbufp", bufs=1) as vbufp, \
             tc.tile_pool(name="ps", bufs=1, space="PSUM") as ps, \
             tc.tile_pool(name="epool", bufs=17) as epool, \
             tc.tile_pool(name="rawp", bufs=2) as rawp, \
             tc.tile_pool(name="xtp", bufs=1) as xtp, \
             tc.tile_pool(name="ropep", bufs=2) as ropep, \
             tc.tile_pool(name="normp", bufs=4) as normp, \
             tc.tile_pool(name="stp", bufs=2) as stp, \
             tc.tile_pool(name="wp", bufs=1) as wp, \
             tc.tile_pool(name="outp", bufs=2) as outp, \
             tc.tile_pool(name="dram", bufs=1, space="DRAM") as dram:

            # ---------- constants ----------
            tri_f = consts.tile([KT, KT], F32)
            make_upper_triangular(nc, tri_f[:], val=1.0, diag=True)
            tri = consts.tile([KT, KT], F16)
            nc.vector.tensor_copy(tri, tri_f)
            ident = consts.tile([128, 128], F32)
            make_identity(nc, ident[:])
            ones16 = consts.tile([128, 16], F16)
            nc.vector.memset(ones16, 1.0)

            # cos/sin tables for both batches (host-precomputed)
            csC = consts.tile([DIMS, BATCH * SEQ], F16, name="csC")
            csS = consts.tile([DIMS, BATCH * SEQ], F16, name="csS")
            nc.sync.dma_start(out=csC, in_=csC_d[:])
            nc.sync.dma_start(out=csS, in_=csS_d[:])

            a2a_in = [dram.tile([NCORES, DIMS, 2 * KT], F16, name=f"a2ain{b}")
                      for b in range(BATCH)]
            a2a_out = [dram.tile([NCORES, DIMS, 2 * KT], F16, name=f"a2aout{b}")
                       for b in range(BATCH)]

            # per-batch persistent tiles
            qR = {b: qkp.tile([DIMS, SEQ], F16, tag=f"qR{b}", name=f"qR{b}")
                  for b in range(BATCH)}
            kR = {b: qkp.tile([DIMS, SEQ], F16, tag=f"kR{b}", name=f"kR{b}")
                  for b in range(BATCH)}
            vbuf = {b: vbufp.tile([128, 130 * (SEQ // KT)], F16, tag=f"vb{b}",
                                  name=f"vbuf{b}")
                    for b in range(BATCH)}

            # projection weights
            w_sb = {}
            for nm, d in (("q", wqT_d), ("k", wkT_d), ("v", wvT_d)):
                wt = wp.tile([128, 8, DIMS], F16, tag=f"w{nm}", name=f"w{nm}")
                nc.sync.dma_start(
                    out=wt, in_=d[:].rearrange("(e p) c -> p e c", p=128))
                w_sb[nm] = [wt[:, k8, :] for k8 in range(8)]

            def emit_wo_loads():
                wo_sb = []
                for k8 in range(8):
                    t = wp.tile([128, D_MODEL], F16, tag=f"wo{k8}",
                                name=f"wo{k8}")
                    nc.sync.dma_start(out=t,
                                      in_=woT_d[k8 * 128:(k8 + 1) * 128, :])
                    wo_sb.append(t)
                return wo_sb

            # whole-x residency: 8 tiles of [128, 4096] f16, loaded once
            xt_all = []
            for k8 in range(8):
                xtile = xtp.tile([128, BATCH * SEQ], F16, tag=f"xt{k8}",
                                 name=f"xt{k8}")
                nc.sync.dma_start(out=xtile, in_=xT_d[k8 * 128:(k8 + 1) * 128, :])
                xt_all.append(xtile)

            # ---------- phase A: one 512-token chunk ----------
            def emit_chunk(b, t):
                g0 = b * SEQ + t * QC          # global column offset
                c0 = t * QC                    # within-batch column offset
                # q,k projections -> one (dim, 2*QC) tile [q | k]
                rqk = rawp.tile([DIMS, 2 * QC], F16, tag="rawqk", name="rawqk")
                for ni, nm in enumerate(("q", "k")):
                    pp = ps.tile([128, QC], F32, tag="m1", bufs=2, name="pp")
                    for k8 in range(8):
                        nc.tensor.matmul(pp, w_sb[nm][k8],
                                         xt_all[k8][:, g0:g0 + QC],
                                         start=(k8 == 0), stop=(k8 == 7))
                    nc.scalar.copy(rqk[:, ni * QC:(ni + 1) * QC], pp)

                # v: projected directly transposed, (token, dim) per 128-tile
                pv = ps.tile([128, QC], F32, tag="m1", bufs=2, name="pv")
                for i in range(NVT):
                    for k8 in range(8):
                        nc.tensor.matmul(
                            pv[:, i * KT:(i + 1) * KT],
                            xt_all[k8][:, g0 + i * KT:g0 + (i + 1) * KT],
                            w_sb["v"][k8],
                            start=(k8 == 0), stop=(k8 == 7),
                            skip_group_check=True)
                vb = vbuf[b]
                cview = vb[:, 130 * NVT * t:130 * NVT * (t + 1)]
                v_view = cview.rearrange("p (n c) -> p n c", c=130)
                for col in (64, 129):
                    nc.vector.tensor_copy(
                        v_view[:, :, col:col + 1].rearrange("p n c -> p (n c)"),
                        ones16[:, 0:NVT])
                dst = cview.rearrange("p (n h c) -> p n h c", h=2, c=65)[
                    :, :, :, 0:64]
                src = pv[:].rearrange("p (n h c) -> p n h c", h=2, c=64)
                nc.vector.tensor_copy(dst, src)

                # RoPE: rot = x*C + swap(x)*S   (sign baked into S rows)
                src = rqk[:].rearrange("(p two) n -> p two n", two=2)
                swp = ropep.tile([DIMS, 2 * QC], F16, tag="swap", name="swp")
                dstv = swp[:].rearrange("(p two) n -> p two n", two=2)
                nc.gpsimd.dma_start(out=dstv[:, 0, :], in_=src[:, 1, :])
                nc.gpsimd.dma_start(out=dstv[:, 1, :], in_=src[:, 0, :])
                for ni, nm in enumerate(("q", "k")):
                    nsl = slice(ni * QC, (ni + 1) * QC)
                    t1 = ropep.tile([DIMS, QC], F16, tag="t1", name="t1")
                    nc.vector.tensor_mul(t1, rqk[:, nsl], csC[:, g0:g0 + QC])
                    nc.vector.tensor_mul(swp[:, nsl], swp[:, nsl],
                                         csS[:, g0:g0 + QC])
                    dst = (qR if nm == "q" else kR)[b][:, c0:c0 + QC]
                    nc.vector.tensor_add(dst, t1, swp[:, nsl])

            # ---------- phase B: one q-chunk of attention ----------
            def emit_attn(b, qc):
                pctx = [ps.tile([128, QC], F32, tag=f"ctx{h}", bufs=1,
                                name=f"pctx{h}")
                        for h in range(H_PER_CORE)]
                nkt = NVT * qc + NVT
                ets = []
                for kt in range(nkt):
                    j = kt - NVT * qc            # >=0: diagonal band tile
                    q0 = 0 if j < 0 else KT * j
                    pl = ps.tile([128, 2 * QC], F32, tag="logit", bufs=2,
                                 name="pl")
                    for h in range(H_PER_CORE):
                        nc.tensor.matmul(
                            pl[:, h * QC + q0:(h + 1) * QC],
                            kR[b][64 * h:64 * (h + 1), kt * KT:(kt + 1) * KT],
                            qR[b][64 * h:64 * (h + 1),
                                  qc * QC + q0:(qc + 1) * QC],
                            start=True, stop=True)
                    et = epool.tile([128, 2 * QC], F16, tag="e", name="et")
                    if q0 == 0:
                        nc.scalar.activation(et, pl, AFT.Exp, scale=SCALE)
                    else:
                        ev = et[:].rearrange("p (h n) -> p h n", h=2)[
                            :, :, q0:QC]
                        pv = pl[:].rearrange("p (h n) -> p h n", h=2)[
                            :, :, q0:QC]
                        nc.scalar.activation(ev, pv, AFT.Exp, scale=SCALE)
                    if j >= 0:
                        for h in range(H_PER_CORE):
                            msl = slice(h * QC + q0, h * QC + q0 + KT)
                            nc.vector.tensor_mul(et[:, msl], et[:, msl], tri)
                    if DEBUG and b == 0 and qc == 0 and kt <= 1:
                        dt_ = normp.tile([128, 2 * QC], F32, tag="dbge",
                                         name="dbge", bufs=1)
                        nc.vector.tensor_copy(dt_, et)
                        nc.sync.dma_start(out=dbg_et[kt], in_=dt_)
                    ets.append(et)
                # AV, one accumulation group per (h, q-tile): only one open
                # group per psum bank at a time (PE constraint)
                for qt in range(NVT):
                    for kt in range(NVT * qc + qt + 1):
                        for h in range(H_PER_CORE):
                            vt = vbuf[b][:, 130 * kt + 65 * h:
                                         130 * kt + 65 * h + 65]
                            nc.tensor.matmul(
                                pctx[h][:, qt * KT:qt * KT + 65],
                                ets[kt][:, h * QC + qt * KT:
                                        h * QC + (qt + 1) * KT],
                                vt,
                                start=(kt == 0),
                                stop=(kt == NVT * qc + qt),
                                skip_group_check=True)

                if DEBUG and b == 0 and qc == 0:
                    dpx = normp.tile([128, QC], F32, tag="dbgp", name="dbgp",
                                     bufs=1)
                    nc.vector.tensor_copy(dpx, pctx[0][:])
                    nc.sync.dma_start(out=dbg_px[:], in_=dpx)

                # normalize, transpose to (dim, token), stage for the A2A
                recips = []
                for h in range(H_PER_CORE):
                    rc = normp.tile([128, NVT], F32, tag="rc", name="rc")
                    den = pctx[h][:].rearrange("p (n c) -> p n c", c=KT)[
                        :, :, 64:65].rearrange("p n c -> p (n c)")
                    nc.vector.reciprocal(rc, den)
                    recips.append(rc)
                stg = stp.tile([128, QC], F16, tag="stage", name="stage")
                for qt in range(NVT):
                    tp = ps.tile([128, QC], F32, tag="m1", bufs=2, name="tp")
                    nr = normp.tile([128, 128], F32, tag="nr", name="nr")
                    for h in range(H_PER_CORE):
                        nc.vector.tensor_scalar_mul(
                            nr[:, 64 * h:64 * (h + 1)],
                            pctx[h][:, qt * KT:qt * KT + 64],
                            recips[h][:, qt:qt + 1])
                    nc.tensor.transpose(tp[:, 0:128], nr, ident[:])
                    nc.scalar.copy(stg[:, qt * KT:(qt + 1) * KT], tp[:, 0:128])
                for half in range(2):
                    nc.sync.dma_start(
                        out=a2a_in[b][2 * qc + half],
                        in_=stg[:, half * 2 * KT:(half + 1) * 2 * KT])

            def emit_a2a(b):
                nc.gpsimd.collective_compute(
                    "AllToAll", mybir.AluOpType.bypass,
                    replica_groups=[list(range(NCORES))],
                    ins=[a2a_in[b].opt()], outs=[a2a_out[b].opt()],
                )

            def emit_woproj(b, wo_sb):
                cm = wp.tile([DIMS, NCORES, 2 * KT], F16, tag=f"cm{b}",
                             name=f"cm{b}")
                nc.sync.dma_start(
                    out=cm, in_=a2a_out[b][:].rearrange("e p c -> p e c"))
                for mt in range(2):
                    ot = outp.tile([128, D_MODEL], F32, tag="out", name="ot")
                    for nn in range(2):
                        po = ps.tile([128, QC], F32, tag="m1", bufs=2,
                                     name="po")
                        for i in range(NCORES):
                            nc.tensor.matmul(
                                po, cm[:, i, mt * KT:(mt + 1) * KT],
                                wo_sb[i][:, nn * QC:(nn + 1) * QC],
                                start=(i == 0), stop=(i == NCORES - 1))
                        nc.scalar.copy(ot[:, nn * QC:(nn + 1) * QC], po)
                    nc.sync.dma_start(out=out_d[b, mt * KT:(mt + 1) * KT, :],
                                      in_=ot)

            # ---------- emission schedule ----------
            emit_chunk(0, 0)
            wo_sb = emit_wo_loads()
            emit_chunk(0, 1)
            emit_attn(0, 0)
            emit_chunk(0, 2)
            emit_attn(0, 1)
            emit_chunk(0, 3)
            emit_attn(0, 2)
            emit_chunk(1, 0)
            emit_attn(0, 3)
            emit_a2a(0)
            emit_chunk(1, 1)
            emit_attn(1, 0)
            emit_chunk(1, 2)
            emit_attn(1, 1)
            emit_chunk(1, 3)
            emit_attn(1, 2)
            emit_woproj(0, wo_sb)
            emit_attn(1, 3)
            emit_a2a(1)
            emit_woproj(1, wo_sb)
            if DEBUG:
                nc.sync.dma_start(out=dbg_qr[:], in_=qR[0])
                nc.sync.dma_start(out=dbg_kr[:], in_=kR[0])
                nc.sync.dma_start(out=dbg_vb[:], in_=vbuf[0])
                cm0 = wp.tile([DIMS, NCORES, 2 * KT], F16, tag="cm0",
                              name="cmdbg")
                nc.sync.dma_start(
                    out=cm0, in_=a2a_out[0][:].rearrange("e p c -> p e c"))
                nc.sync.dma_start(
                    out=dbg_st[:].rearrange("e p c -> p e c"), in_=cm0)

    nc.compile()
    return nc


def _host_prep(inputs):
    x = np.asarray(inputs["in_features"], dtype=np.float32)
    tp = np.asarray(inputs["token_positions"], dtype=np.int32)
    wq = np.asarray(inputs["wq"], dtype=np.float32)
    wk = np.asarray(inputs["wk"], dtype=np.float32)
    wv = np.asarray(inputs["wv"], dtype=np.float32)
    wo = np.asarray(inputs["wo"], dtype=np.float32)

    xT = np.ascontiguousarray(
        np.concatenate([x[b].T for b in range(BATCH)], axis=1)).astype(np.float16)
    woT = np.ascontiguousarray(wo.T).astype(np.float16)

    # cos/sin tables, (dim row, batch*token col); sign baked into S so that
    # rot = x*C + swap(x)*S
    half = D_K // 2
    inv_freq = 1.0 / (THETA ** (2.0 * np.arange(half) / D_K))     # (32,)
    ang = tp.astype(np.float64)[:, :, None] * inv_freq[None, None, :]
    cos = np.cos(ang)                                             # (B, S, 32)
    sin = np.sin(ang)
    rows = np.arange(DIMS)
    j = (rows % D_K) // 2                                         # freq index
    sign = np.where(rows % 2 == 0, -1.0, 1.0)
    csC = np.empty((DIMS, BATCH * SEQ), dtype=np.float16)
    csS = np.empty((DIMS, BATCH * SEQ), dtype=np.float16)
    for b in range(BATCH):
        csC[:, b * SEQ:(b + 1) * SEQ] = cos[b][:, j].T
        csS[:, b * SEQ:(b + 1) * SEQ] = (sin[b][:, j] * sign[None, :]).T

    in_maps = []
    for c in range(NCORES):
        rsl = slice(DIMS * c, DIMS * (c + 1))
        in_maps.append({
            "xT": xT,
            "wqT": np.ascontiguousarray(wq[rsl].T).astype(np.float16),
            "wkT": np.ascontiguousarray(wk[rsl].T).astype(np.float16),
            "wvT": np.ascontiguousarray(wv[rsl].T).astype(np.float16),
            "woT": woT,
            "csC": csC,
            "csS": csS,
        })
    return in_maps


def kernel(**inputs) -> np.ndarray:
    from concourse.bass_utils import run_bass_kernel_spmd

    if "nc" not in _CACHE:
        _CACHE["nc"] = _build_program()
    nc = _CACHE["nc"]

    in_maps = _host_prep(inputs)
    res = run_bass_kernel_spmd(nc, in_maps, list(range(NCORES))).results

    out = np.empty((BATCH, SEQ, D_MODEL), dtype=np.float32)
    for c in range(NCORES):
        qc, hf = c // 2, c % 2
        t0 = qc * QC + hf * 2 * KT
        for b in range(BATCH):
            out[b, t0:t0 + 2 * KT, :] = res[c]["out"][b]
    return out
